# revision 2
# baseline (speedup 1.0000x reference)
"""TRN2 Bass kernel for nn_CrossAttentionHeightSplit.

Computation: 26-view cross-attention. For each scene b (2) and view i (26):
  q = x[b,i] (1024 tokens, C=256), kv = concat of x[b, sel(i)] neighbors
  (3-4 views, 1024 tokens each), 8-head MHA with weight group mha_index(i).

Sharding: the 52 (b, view) attention problems are split in half along the
query-token axis into 104 half-tasks (512 q-tokens each). Each of the 8
cores gets exactly 9 four-neighbor and 4 three-neighbor half-tasks
(perfectly balanced, identical static program on every core = SPMD).
Weights are gathered per-slot on the host (replicated as needed).

On-core dataflow per slot (all layouts channel-major [c, tokens], which
matches x's HBM layout directly):
  qpT = WqT.T @ xq           [256, 512]  (fp32r matmuls, bf16 result)
  kpT = WkT.T @ xn           [256, n*1024] bf16
  v   = xn.T @ WvT           [kv-tokens on partitions, 264 = 8 heads x 33]
        (per head: 32 v-dims + a ones column -> av matmul also produces
         the softmax denominator for free)
  per (head, neighbor): scoresT[kv,q] on PE (K=32 slice), exp on ACT
        (6 of 8 kv-tiles staged via DVE copy + one batched in-place exp,
         2 exp'd directly from PSUM by ACT - balances DVE/ACT load),
  av[33, 512] accumulated over all kv chunks in PSUM,
  normalize via DVE reciprocal + GPSIMD partition-broadcast + DVE mul,
  oT = WoT.T @ avnT + bo     [256, 512]  == output layout, DMA'd out.
"""

import sys
import numpy as np

try:
    import concourse.bass as bass  # noqa: F401
except ImportError:
    sys.path.insert(0, "/opt/trn_rl_repo")

import concourse.bacc as bacc
import concourse.mybir as mybir
import concourse.tile as tile
from concourse.bass_utils import run_bass_kernel_spmd

dt = mybir.dt
AF = mybir.ActivationFunctionType

# ---------------------------------------------------------------- constants
N_VIEWS = 26
C = 256
S = 1024          # tokens per view
SH = 512          # tokens per half-task
NH = 8            # heads
D = 32            # head dim
ISQ = float(1.0 / np.sqrt(D))

# neighbor selection (angular-distance graph from the reference model)
SEL = {
    0: [18, 20, 22, 24], 1: [2, 4, 6, 8], 2: [1, 3, 9, 10], 3: [2, 4, 11],
    4: [1, 3, 5, 12], 5: [4, 6, 13], 6: [1, 5, 7, 14], 7: [6, 8, 15],
    8: [1, 7, 9, 16], 9: [2, 8, 17], 10: [2, 11, 17, 18], 11: [3, 10, 12, 19],
    12: [4, 11, 13, 20], 13: [5, 12, 14, 21], 14: [6, 13, 15, 22],
    15: [7, 14, 16, 23], 16: [8, 15, 17, 24], 17: [9, 10, 16, 25],
    18: [0, 10, 19, 25], 19: [11, 18, 20], 20: [0, 12, 19, 21],
    21: [13, 20, 22], 22: [0, 14, 21, 23], 23: [15, 22, 24],
    24: [0, 16, 23, 25], 25: [17, 18, 24],
}
MHA_IDX = [0, 1] + [2] * 8 + [3] * 8 + [4] * 8

N_CORES = 8
SLOT_N = [4] * 9 + [3] * 4       # neighbors per slot; identical on all cores
N_SLOTS = len(SLOT_N)            # 13
KV_ROWS = sum(SLOT_N)            # 48
KVOFF = np.concatenate([[0], np.cumsum(SLOT_N)]).astype(int)

# half-task assignment: task = (b, view, qhalf)
_V4 = [i for i in range(N_VIEWS) if len(SEL[i]) == 4]   # 18 views
_V3 = [i for i in range(N_VIEWS) if len(SEL[i]) == 3]   # 8 views
_T4 = [(b, i, h) for b in range(2) for i in _V4 for h in range(2)]  # 72
_T3 = [(b, i, h) for b in range(2) for i in _V3 for h in range(2)]  # 32
ASSIGN = [ _T4[c * 9:(c + 1) * 9] + _T3[c * 4:(c + 1) * 4] for c in range(N_CORES) ]

# how many of the 4 score psum pairs per (head, nbr) are evacuated by DVE
# (rest exp'd directly by ACT); measurement shows ACT runs at 1 elem/lane/cycle
# regardless of dtype, so DVE staging adds work without reducing ACT time -> 0
DVE_PAIRS = 0

_PROGRAM_CACHE = {}


def _build_program():
    """Build + compile the SPMD Tile program (identical on all cores)."""
    if "nc" in _PROGRAM_CACHE:
        return _PROGRAM_CACHE["nc"]

    nc = bacc.Bacc("TRN2", target_bir_lowering=False, debug=False)

    xq_d = nc.dram_tensor("xq", [N_SLOTS, C, SH], dt.float32, kind="ExternalInput").ap()
    xkv_d = nc.dram_tensor("xkv", [KV_ROWS, C, S], dt.float32, kind="ExternalInput").ap()
    wqkvT_d = nc.dram_tensor("wqkvT", [N_SLOTS, C, 3 * C], dt.float32, kind="ExternalInput").ap()
    woT_d = nc.dram_tensor("woT", [N_SLOTS, C, C], dt.float32, kind="ExternalInput").ap()
    bqkv_d = nc.dram_tensor("bqkv", [N_SLOTS, 3 * C, 1], dt.float32, kind="ExternalInput").ap()
    bo_d = nc.dram_tensor("bo", [N_SLOTS, C, 1], dt.float32, kind="ExternalInput").ap()
    out_d = nc.dram_tensor("out", [N_SLOTS, C, SH], dt.float32, kind="ExternalOutput").ap()

    f32, f32r, bf16 = dt.float32, dt.float32r, dt.bfloat16

    from contextlib import ExitStack
    with ExitStack() as stack:
        tc = stack.enter_context(tile.TileContext(nc))
        wp = stack.enter_context(tc.tile_pool(name="wp", bufs=4))
        wop = stack.enter_context(tc.tile_pool(name="wop", bufs=4))
        biasp = stack.enter_context(tc.tile_pool(name="biasp", bufs=16))
        xqp = stack.enter_context(tc.tile_pool(name="xqp", bufs=4))
        xnp = stack.enter_context(tc.tile_pool(name="xnp", bufs=3))
        qp_pool = stack.enter_context(tc.tile_pool(name="qp", bufs=4))
        kp_pool = stack.enter_context(tc.tile_pool(name="kp", bufs=4))
        vp_pool = stack.enter_context(tc.tile_pool(name="vp", bufs=2))
        esp = stack.enter_context(tc.tile_pool(name="esp", bufs=4))
        avp = stack.enter_context(tc.tile_pool(name="avp", bufs=4))
        otp = stack.enter_context(tc.tile_pool(name="otp", bufs=4))
        recp = stack.enter_context(tc.tile_pool(name="recp", bufs=2))
        rbp = stack.enter_context(tc.tile_pool(name="rbp", bufs=2))
        psc = stack.enter_context(tc.tile_pool(name="psc", bufs=2, space="PSUM"))
        pav_pool = stack.enter_context(tc.tile_pool(name="pav", bufs=2, space="PSUM"))
        ppr = stack.enter_context(tc.tile_pool(name="ppr", bufs=2, space="PSUM"))

        if True:
            for t in range(N_SLOTS):
                n = SLOT_N[t]

                # ---- load weights / biases for this slot
                w_sb = []
                wo_sb = []
                for ki in range(2):
                    w = wp.tile([128, 3 * C], f32r, tag="w")
                    nc.sync.dma_start(w, wqkvT_d[t, ki * 128:(ki + 1) * 128, :].bitcast(f32r))
                    w_sb.append(w)
                    wo = wop.tile([128, C], f32r, tag="wo")
                    nc.sync.dma_start(wo, woT_d[t, ki * 128:(ki + 1) * 128, :].bitcast(f32r))
                    wo_sb.append(wo)
                bq, bk, bv, bo = [], [], [], []
                for mo in range(2):
                    for lst, base, src in ((bq, 0, bqkv_d), (bk, C, bqkv_d), (bv, 2 * C, bqkv_d)):
                        b_ = biasp.tile([128, 1], f32, tag="bias")
                        nc.sync.dma_start(b_, src[t, base + mo * 128: base + (mo + 1) * 128, :])
                        lst.append(b_)
                    b_ = biasp.tile([128, 1], f32, tag="bias")
                    nc.sync.dma_start(b_, bo_d[t, mo * 128:(mo + 1) * 128, :])
                    bo.append(b_)

                # ---- load q-half and project: qpT [2][128, SH] bf16
                xq_sb = []
                for ki in range(2):
                    xq = xqp.tile([128, SH], f32r, tag="xq")
                    nc.sync.dma_start(xq, xq_d[t, ki * 128:(ki + 1) * 128, :].bitcast(f32r))
                    xq_sb.append(xq)
                qpT = []
                for mo in range(2):
                    pq = ppr.tile([128, 512], f32, tag="proj")
                    for ki in range(2):
                        nc.tensor.matmul(pq[:, 0:SH], w_sb[ki][:, mo * 128:(mo + 1) * 128],
                                         xq_sb[ki], start=(ki == 0), stop=(ki == 1))
                    q_bf = qp_pool.tile([128, SH], bf16, tag="qpT")
                    nc.vector.tensor_scalar_add(q_bf, pq[:, 0:SH], bq[mo])
                    qpT.append(q_bf)

                # ---- per-neighbor K/V projection
                kpT = [kp_pool.tile([128, 4 * S], bf16, tag="kpT", name=f"kpT{_mo}") for _mo in range(2)]
                v_sb = vp_pool.tile([128, 32 * 264], bf16, tag="v")
                # ones columns for the softmax-denominator trick (all at once)
                nc.vector.memset(
                    v_sb.rearrange("p (g h e) -> p g h e", h=NH, e=D + 1)[:, :, :, D:D + 1], 1.0)

                for j in range(n):
                    xn_sb = []
                    for ki in range(2):
                        xn = xnp.tile([128, S], f32r, tag="xn")
                        nc.sync.dma_start(xn, xkv_d[KVOFF[t] + j, ki * 128:(ki + 1) * 128, :].bitcast(f32r))
                        xn_sb.append(xn)
                    # kpT
                    for mo in range(2):
                        for nq in range(2):
                            pk = ppr.tile([128, 512], f32, tag="proj")
                            for ki in range(2):
                                nc.tensor.matmul(pk, w_sb[ki][:, C + mo * 128: C + (mo + 1) * 128],
                                                 xn_sb[ki][:, nq * 512:(nq + 1) * 512],
                                                 start=(ki == 0), stop=(ki == 1))
                            nc.vector.tensor_scalar_add(
                                kpT[mo][:, j * S + nq * 512: j * S + (nq + 1) * 512], pk, bk[mo])
                    # v (transposed layout: kv tokens on partitions)
                    for st in range(8):
                        pv = ppr.tile([128, 512], f32, tag="proj")
                        for ki in range(2):
                            nc.tensor.matmul(pv[:, 0:C], xn_sb[ki][:, st * 128:(st + 1) * 128],
                                             w_sb[ki][:, 2 * C:3 * C], start=(ki == 0), stop=(ki == 1))
                        g = j * 8 + st
                        dst = v_sb[:, g * 264:(g + 1) * 264].rearrange(
                            "p (h e) -> p h e", e=D + 1)[:, :, 0:D]
                        nc.vector.tensor_copy(dst, pv[:, 0:C].rearrange("p (h d) -> p h d", d=D))

                # ---- attention
                # scores: per-head blocks of same tile-position matmuls
                # (adjacent different row-group positions are numerically
                # broken on this toolchain - probe-verified). av: the two
                # heads of a pair accumulate into one [97, SH] psum tile on
                # alternating 64-col PE groups (verified ~2x overlap);
                # rows 0:33 head-even, 64:97 head-odd.
                avnT = [avp.tile([128, SH], f32r, tag="avnT", name=f"avnT{_mo}") for _mo in range(2)]
                for pr in range(4):
                    qtile = qpT[pr // 2]
                    ktile = kpT[pr // 2]
                    pav2 = pav_pool.tile([97, SH], f32, tag="av", name=f"pav_{t}_{pr}")
                    for j in range(n):
                        es2 = [esp.tile([128, 8 * 512], bf16, tag="es",
                                        name=f"es_{t}_{pr}_{j}_{hh}") for hh in range(2)]
                        for hh in range(2):
                            h = 2 * pr + hh
                            hp = (h % 4) * 32
                            for cp in range(4):
                                pss = psc.tile([128, 1024], f32, tag="sc")
                                for u in range(2):
                                    c = cp * 2 + u
                                    nc.tensor.matmul(
                                        pss[:, u * 512:(u + 1) * 512],
                                        ktile[hp:hp + 32, j * S + c * 128: j * S + (c + 1) * 128],
                                        qtile[hp:hp + 32, :], start=True, stop=True,
                                        tile_position=(hp, 0))
                                if cp < DVE_PAIRS:
                                    nc.vector.tensor_copy(
                                        es2[hh][:, cp * 1024:(cp + 1) * 1024], pss)
                                else:
                                    nc.scalar.activation(
                                        es2[hh][:, cp * 1024:(cp + 1) * 1024], pss,
                                        AF.Exp, scale=ISQ)
                            if DVE_PAIRS > 0:
                                sl = es2[hh][:, 0:DVE_PAIRS * 1024]
                                nc.scalar.activation(sl, sl, AF.Exp, scale=ISQ)
                        for c in range(8):
                            g = j * 8 + c
                            st_, sp_ = (j == 0 and c == 0), (j == n - 1 and c == 7)
                            for hh in range(2):
                                h = 2 * pr + hh
                                rows = pav2[0:33, :] if hh == 0 else pav2[64:97, :]
                                cg = 0 if hh == 0 else 64
                                nc.tensor.matmul(
                                    rows, v_sb[:, g * 264 + 33 * h: g * 264 + 33 * h + 33],
                                    es2[hh][:, c * 512:(c + 1) * 512],
                                    start=st_, stop=sp_, tile_position=(0, cg))
                    # normalize the pair's 2 heads
                    for hh in range(2):
                        h = 2 * pr + hh
                        sums_row = pav2[32:33, :] if hh == 0 else pav2[96:97, :]
                        av_rows = pav2[0:32, :] if hh == 0 else pav2[64:96, :]
                        srow = recp.tile([1, SH], f32, tag="rec")
                        nc.vector.tensor_copy(srow, sums_row)
                        rec = recp.tile([1, SH], f32, tag="rec2")
                        nc.vector.reciprocal_approx_fast(rec, srow)
                        rb = rbp.tile([32, SH], f32, tag="rb")
                        nc.gpsimd.partition_broadcast(rb, rec)
                        nc.vector.tensor_mul(avnT[pr // 2][(h % 4) * 32:(h % 4) * 32 + 32, :],
                                             av_rows, rb)

                # ---- v-bias (zero in practice, but general) + out-projection
                for mo in range(2):
                    nc.vector.tensor_scalar_add(avnT[mo], avnT[mo], bv[mo])
                for mo in range(2):
                    po = ppr.tile([128, 512], f32, tag="proj")
                    for ki in range(2):
                        nc.tensor.matmul(po[:, 0:SH], wo_sb[ki][:, mo * 128:(mo + 1) * 128],
                                         avnT[ki], start=(ki == 0), stop=(ki == 1))
                    oT = otp.tile([128, SH], f32, tag="oT")
                    nc.vector.tensor_scalar_add(oT, po[:, 0:SH], bo[mo])
                    nc.sync.dma_start(out_d[t, mo * 128:(mo + 1) * 128, :], oT)

    nc.compile()
    _PROGRAM_CACHE["nc"] = nc
    return nc


def _prep_inputs(x, w_qkv, b_qkv, w_out, b_out):
    x = np.ascontiguousarray(np.asarray(x, dtype=np.float32))
    w_qkv = np.asarray(w_qkv, dtype=np.float32)
    b_qkv = np.asarray(b_qkv, dtype=np.float32)
    w_out = np.asarray(w_out, dtype=np.float32)
    b_out = np.asarray(b_out, dtype=np.float32)

    x2 = x.reshape(2, N_VIEWS, C, S)
    in_maps = []
    for core in range(N_CORES):
        tasks = ASSIGN[core]
        xq = np.empty((N_SLOTS, C, SH), np.float32)
        xkv = np.empty((KV_ROWS, C, S), np.float32)
        wqkvT = np.empty((N_SLOTS, C, 3 * C), np.float32)
        woT = np.empty((N_SLOTS, C, C), np.float32)
        bqkv = np.empty((N_SLOTS, 3 * C, 1), np.float32)
        bo = np.empty((N_SLOTS, C, 1), np.float32)
        for t, (b, i, qh) in enumerate(tasks):
            m = MHA_IDX[i]
            xq[t] = x2[b, i][:, qh * SH:(qh + 1) * SH]
            for j, nb in enumerate(SEL[i]):
                xkv[KVOFF[t] + j] = x2[b, nb]
            wqkvT[t] = w_qkv[m].T
            woT[t] = w_out[m].T
            bqkv[t, :, 0] = b_qkv[m]
            bo[t, :, 0] = b_out[m]
        in_maps.append({
            "xq": xq, "xkv": xkv, "wqkvT": wqkvT, "woT": woT,
            "bqkv": bqkv, "bo": bo,
        })
    return in_maps


def _gather_output(results, dtype):
    y = np.empty((2, N_VIEWS, C, S), np.float32)
    for core in range(N_CORES):
        out = results[core]["out"]
        for t, (b, i, qh) in enumerate(ASSIGN[core]):
            y[b, i][:, qh * SH:(qh + 1) * SH] = out[t]
    return y.reshape(2 * N_VIEWS, C, 32, 32).astype(dtype, copy=False)


def _run(inputs, trace=False, tmpdir=None):
    nc = _build_program()
    in_maps = _prep_inputs(**inputs)
    res = run_bass_kernel_spmd(nc, in_maps, core_ids=list(range(N_CORES)),
                               trace=trace, tmpdir=tmpdir)
    y = _gather_output(res.results, np.asarray(inputs["x"]).dtype)
    return y, res


def kernel(x, w_qkv, b_qkv, w_out, b_out):
    y, _ = _run(dict(x=x, w_qkv=w_qkv, b_qkv=b_qkv, w_out=w_out, b_out=b_out))
    return y



# revision 5
# speedup vs baseline: 1.0596x; 1.0596x over previous
"""TRN2 Bass kernel for nn_CrossAttentionHeightSplit (v2).

Computation: 26-view cross-attention. For each scene b (2) and view i (26):
  q = x[b,i] (1024 tokens, C=256), kv = concat of x[b, sel(i)] neighbors
  (3-4 views, 1024 tokens each), 8-head MHA with weight group mha_index(i).

v2 design notes (vs v1):
- Tasks are FULL views (q=1024), so each neighbor's K/V projection is
  computed once (26 kv-rows/core instead of 48). Per core: 4 full 4-nbr
  views + 1 half 4-nbr view + 2 full 3-nbr views = identical SPMD program,
  perfectly balanced (6656 q tokens, 24 full attention pairs each).
- All inputs pre-cast to bf16 on the host: halves DMA and runs matmuls
  at full bf16 PE rate (fp32 moving data streams ~2x slower).
- exp always runs directly PSUM->SBUF on ACT (FD=1024): ACT is a
  1 elem/lane/cycle LUT pipeline regardless of dtype, so DVE staging of
  scores is pure overhead (measured). ACT is the roofline engine here:
  201M score elements/core ~= 1.3 ms minimum.
- k-bias dropped (softmax is invariant to per-q score shifts: (q+bq)@bk
  is constant over kv) and v-bias folded into the output bias on the host
  (bo_eff = bo + Wo@bv). Both exact. All PSUM evacuations are single
  fused DVE ops.
- Attention processes heads in quads: 4 concurrent score matmuls on 4
  distinct PE row groups (tile_position), with two row-group outputs
  sharing one [128,1024] psum tile via different 512-col halves
  (= different banks), so exp stays FD=1024. Hardware-verified correct
  (bit-identical to the serial order).
- PSUM: 3 rotating [128,1024] score/proj tiles + 2 [97,512] av
  accumulators = exactly 8 banks. 3 score tiles in flight keep the PE
  queue runnable ahead of ACT; with only 2, every matmul paid isolated
  cold-issue latency and HAM kept the PE throttled at 1.2 GHz.
- av matmuls are software-pipelined one kv-chunk behind the scores so
  the PE issues scores(c) + av(c-1) back-to-back without waiting on
  ACT, and alternate PE col groups for 2-way concurrency.
"""

import sys
import numpy as np

try:
    import concourse.bass as bass  # noqa: F401
except ImportError:
    sys.path.insert(0, "/opt/trn_rl_repo")

import ml_dtypes
import concourse.bacc as bacc
import concourse.mybir as mybir
import concourse.tile as tile
from concourse.bass_utils import run_bass_kernel_spmd

dt = mybir.dt
AF = mybir.ActivationFunctionType
BF16 = ml_dtypes.bfloat16

# ---------------------------------------------------------------- constants
N_VIEWS = 26
C = 256
S = 1024          # tokens per view
NH = 8            # heads
D = 32            # head dim
ISQ = float(1.0 / np.sqrt(D))

# neighbor selection (angular-distance graph from the reference model)
SEL = {
    0: [18, 20, 22, 24], 1: [2, 4, 6, 8], 2: [1, 3, 9, 10], 3: [2, 4, 11],
    4: [1, 3, 5, 12], 5: [4, 6, 13], 6: [1, 5, 7, 14], 7: [6, 8, 15],
    8: [1, 7, 9, 16], 9: [2, 8, 17], 10: [2, 11, 17, 18], 11: [3, 10, 12, 19],
    12: [4, 11, 13, 20], 13: [5, 12, 14, 21], 14: [6, 13, 15, 22],
    15: [7, 14, 16, 23], 16: [8, 15, 17, 24], 17: [9, 10, 16, 25],
    18: [0, 10, 19, 25], 19: [11, 18, 20], 20: [0, 12, 19, 21],
    21: [13, 20, 22], 22: [0, 14, 21, 23], 23: [15, 22, 24],
    24: [0, 16, 23, 25], 25: [17, 18, 24],
}
MHA_IDX = [0, 1] + [2] * 8 + [3] * 8 + [4] * 8

N_CORES = 8
# slot layout (identical on every core): 4 full 4-nbr views, one half of a
# 4-nbr view, 2 full 3-nbr views
SLOT_N = [4, 4, 4, 4, 4, 3, 3]
SLOT_Q = [S, S, S, S, S // 2, S, S]
N_SLOTS = len(SLOT_N)            # 7
KV_ROWS = sum(SLOT_N)            # 26
KVOFF = np.concatenate([[0], np.cumsum(SLOT_N)]).astype(int)

# task assignment: (b, view, qh) with qh None = full view
_V4 = [i for i in range(N_VIEWS) if len(SEL[i]) == 4]   # 18 views
_V3 = [i for i in range(N_VIEWS) if len(SEL[i]) == 3]   # 8 views
_V4I = [(b, i) for b in range(2) for i in _V4]          # 36 instances
_V3I = [(b, i) for b in range(2) for i in _V3]          # 16 instances


def _assign(core):
    fulls4 = _V4I[core * 4:(core + 1) * 4]
    hb, hi = _V4I[32 + core // 2]
    half = (hb, hi, core % 2)
    fulls3 = _V3I[core * 2:(core + 1) * 2]
    return ([(b, i, None) for (b, i) in fulls4] + [half]
            + [(b, i, None) for (b, i) in fulls3])


ASSIGN = [_assign(c) for c in range(N_CORES)]

_PROGRAM_CACHE = {}


def _build_program():
    """Build + compile the SPMD Tile program (identical on all cores)."""
    if "nc" in _PROGRAM_CACHE:
        return _PROGRAM_CACHE["nc"]

    nc = bacc.Bacc("TRN2", target_bir_lowering=False, debug=False)

    xq_d = nc.dram_tensor("xq", [N_SLOTS, C, S], dt.bfloat16, kind="ExternalInput").ap()
    xkv_d = nc.dram_tensor("xkv", [KV_ROWS, C, S], dt.bfloat16, kind="ExternalInput").ap()
    wqkvT_d = nc.dram_tensor("wqkvT", [N_SLOTS, C, 3 * C], dt.bfloat16, kind="ExternalInput").ap()
    woT_d = nc.dram_tensor("woT", [N_SLOTS, C, C], dt.bfloat16, kind="ExternalInput").ap()
    bq_d = nc.dram_tensor("bq", [N_SLOTS, C, 1], dt.float32, kind="ExternalInput").ap()
    bo_d = nc.dram_tensor("bo", [N_SLOTS, C, 1], dt.float32, kind="ExternalInput").ap()
    out_d = nc.dram_tensor("out", [N_SLOTS, C, S], dt.float32, kind="ExternalOutput").ap()

    f32, bf16 = dt.float32, dt.bfloat16

    from contextlib import ExitStack
    with ExitStack() as stack:
        tc = stack.enter_context(tile.TileContext(nc))
        wp = stack.enter_context(tc.tile_pool(name="wp", bufs=4))
        wop = stack.enter_context(tc.tile_pool(name="wop", bufs=4))
        biasp = stack.enter_context(tc.tile_pool(name="biasp", bufs=8))
        xqp = stack.enter_context(tc.tile_pool(name="xqp", bufs=4))
        xnp = stack.enter_context(tc.tile_pool(name="xnp", bufs=3))
        qp_pool = stack.enter_context(tc.tile_pool(name="qp", bufs=4))
        kp_pool = stack.enter_context(tc.tile_pool(name="kp", bufs=4))
        vp_pool = stack.enter_context(tc.tile_pool(name="vp", bufs=2))
        esp = stack.enter_context(tc.tile_pool(name="esp", bufs=8))
        avp = stack.enter_context(tc.tile_pool(name="avp", bufs=4))
        otp = stack.enter_context(tc.tile_pool(name="otp", bufs=4))
        recp = stack.enter_context(tc.tile_pool(name="recp", bufs=4))
        rbp = stack.enter_context(tc.tile_pool(name="rbp", bufs=4))
        # PSUM: pp [128,1024]x3 = 6 banks (scores + projections), pav
        # [97,512]x2 = 2 banks (av accumulators for the two head-pairs of
        # the active quad) -> 8 banks total.
        # 3 score tiles in flight keep the PE queue runnable ahead of ACT
        # (2 bufs starved it: every MM paid isolated cold-issue latency and
        # HAM kept the PE throttled at 1.2 GHz ~96% of the run).
        pp = stack.enter_context(tc.tile_pool(name="pp", bufs=3, space="PSUM"))
        pav_pool = stack.enter_context(tc.tile_pool(name="pav", bufs=2, space="PSUM"))

        for t in range(N_SLOTS):
            n = SLOT_N[t]
            Q = SLOT_Q[t]

            # ---- load weights / biases for this slot
            w_sb = []
            wo_sb = []
            for ki in range(2):
                w = wp.tile([128, 3 * C], bf16, tag="w")
                nc.sync.dma_start(w, wqkvT_d[t, ki * 128:(ki + 1) * 128, :])
                w_sb.append(w)
                wo = wop.tile([128, C], bf16, tag="wo")
                nc.sync.dma_start(wo, woT_d[t, ki * 128:(ki + 1) * 128, :])
                wo_sb.append(wo)
            bq, bo = [], []
            for mo in range(2):
                b_ = biasp.tile([128, 1], f32, tag="bias")
                nc.sync.dma_start(b_, bq_d[t, mo * 128:(mo + 1) * 128, :])
                bq.append(b_)
                b_ = biasp.tile([128, 1], f32, tag="bias")
                nc.sync.dma_start(b_, bo_d[t, mo * 128:(mo + 1) * 128, :])
                bo.append(b_)

            # ---- load q and project: qpT [2][128, Q] bf16
            xq_sb = []
            for ki in range(2):
                xq = xqp.tile([128, S], bf16, tag="xq")
                nc.sync.dma_start(xq[:, 0:Q], xq_d[t, ki * 128:(ki + 1) * 128, 0:Q])
                xq_sb.append(xq)
            qpT = []
            for mo in range(2):
                pq = pp.tile([128, 1024], f32, tag="ps")
                for u in range(0, Q, 512):
                    for ki in range(2):
                        nc.tensor.matmul(pq[:, u:u + 512], w_sb[ki][:, mo * 128:(mo + 1) * 128],
                                         xq_sb[ki][:, u:u + 512], start=(ki == 0), stop=(ki == 1))
                q_bf = qp_pool.tile([128, S], bf16, tag="qpT")
                nc.vector.tensor_scalar_add(q_bf[:, 0:Q], pq[:, 0:Q], bq[mo])
                qpT.append(q_bf)

            # ---- per-neighbor K/V projection (k-bias dropped: softmax-
            # invariant; v-bias folded into bo on the host)
            kpT = [kp_pool.tile([128, 4 * S], bf16, tag="kpT", name=f"kpT{_mo}") for _mo in range(2)]
            v_sb = vp_pool.tile([128, 32 * 264], bf16, tag="v")
            nc.vector.memset(
                v_sb.rearrange("p (g h e) -> p g h e", h=NH, e=D + 1)[:, :, :, D:D + 1], 1.0)

            for j in range(n):
                xn_sb = []
                for ki in range(2):
                    xn = xnp.tile([128, S], bf16, tag="xn")
                    nc.sync.dma_start(xn, xkv_d[KVOFF[t] + j, ki * 128:(ki + 1) * 128, :])
                    xn_sb.append(xn)
                # kpT
                for mo in range(2):
                    pk = pp.tile([128, 1024], f32, tag="ps")
                    for u in range(0, S, 512):
                        for ki in range(2):
                            nc.tensor.matmul(pk[:, u:u + 512],
                                             w_sb[ki][:, C + mo * 128: C + (mo + 1) * 128],
                                             xn_sb[ki][:, u:u + 512],
                                             start=(ki == 0), stop=(ki == 1))
                    nc.vector.tensor_copy(kpT[mo][:, j * S:(j + 1) * S], pk)
                # v (transposed layout: kv tokens on partitions)
                for st in range(8):
                    pv = pp.tile([128, 1024], f32, tag="ps")
                    for ki in range(2):
                        nc.tensor.matmul(pv[:, 0:C], xn_sb[ki][:, st * 128:(st + 1) * 128],
                                         w_sb[ki][:, 2 * C:3 * C], start=(ki == 0), stop=(ki == 1))
                    g = j * 8 + st
                    dst = v_sb[:, g * 264:(g + 1) * 264].rearrange(
                        "p (h e) -> p h e", e=D + 1)[:, :, 0:D]
                    nc.vector.tensor_copy(dst, pv[:, 0:C].rearrange("p (h d) -> p h d", d=D))

            # ---- attention, heads in quads (4 distinct PE row groups ->
            # 4-way concurrent score MMs), q processed per 512-token half.
            # Two concurrent row-group MMs share one [128,1024] psum tile by
            # writing different 512-col halves (different banks), keeping
            # exp at FD=1024.
            avnT = [avp.tile([128, S], bf16, tag="avnT", name=f"avnT{_mo}") for _mo in range(2)]
            for qd in range(2):
                qtile = qpT[qd]
                ktile = kpT[qd]
                for u in range(0, Q, 512):
                    pav = [pav_pool.tile([97, 512], f32, tag="av",
                                         name=f"pav_{t}_{qd}_{u}_{pp_}") for pp_ in range(2)]

                    def _emit_av(es2, g, st_, sp_):
                        # av: alternate PE col groups -> 2-way concurrency
                        for hh in range(4):
                            h = 4 * qd + hh
                            cg = 0 if hh % 2 == 0 else 64
                            rows = (pav[hh // 2][0:33, :] if hh % 2 == 0
                                    else pav[hh // 2][64:97, :])
                            nc.tensor.matmul(
                                rows,
                                v_sb[:, g * 264 + 33 * h: g * 264 + 33 * h + 33],
                                es2[hh // 2][:, (hh % 2) * 512:(hh % 2) * 512 + 512],
                                start=st_, stop=sp_, tile_position=(0, cg))

                    # software pipeline: av runs one chunk behind the scores
                    # so the PE issues scores(c) + av(c-1) back-to-back with
                    # no ACT-wait in between (denser bursts keep HAM warm)
                    pending = None
                    for j in range(n):
                        for c in range(8):
                            g = j * 8 + c
                            st_, sp_ = (j == 0 and c == 0), (j == n - 1 and c == 7)
                            ps2 = [pp.tile([128, 1024], f32, tag="ps",
                                           name=f"ps_{t}_{qd}_{u}_{j}_{c}_{i_}")
                                   for i_ in range(2)]
                            for hh in range(4):
                                hp = hh * 32
                                nc.tensor.matmul(
                                    ps2[hh // 2][:, (hh % 2) * 512:(hh % 2) * 512 + 512],
                                    ktile[hp:hp + 32, j * S + c * 128: j * S + (c + 1) * 128],
                                    qtile[hp:hp + 32, u:u + 512], start=True, stop=True,
                                    tile_position=(hp, 0))
                            es2 = []
                            for half in range(2):
                                es = esp.tile([128, 1024], bf16, tag="es")
                                nc.scalar.activation(es, ps2[half], AF.Exp, scale=ISQ)
                                es2.append(es)
                            if pending is not None:
                                _emit_av(*pending)
                            pending = (es2, g, st_, sp_)
                    _emit_av(*pending)
                    # normalize the quad's 4 heads for this q-half
                    for hh in range(4):
                        h = 4 * qd + hh
                        pv2 = pav[hh // 2]
                        sums_row = pv2[32:33, :] if hh % 2 == 0 else pv2[96:97, :]
                        av_rows = pv2[0:32, :] if hh % 2 == 0 else pv2[64:96, :]
                        srow = recp.tile([1, 512], f32, tag="rec")
                        nc.vector.tensor_copy(srow, sums_row)
                        rec = recp.tile([1, 512], f32, tag="rec2")
                        nc.vector.reciprocal_approx_fast(rec, srow)
                        rb = rbp.tile([32, 512], f32, tag="rb")
                        nc.gpsimd.partition_broadcast(rb, rec)
                        nc.vector.tensor_mul(avnT[qd][hh * 32:hh * 32 + 32, u:u + 512],
                                             av_rows, rb)

            # ---- out-projection (v-bias already folded into bo)
            for mo in range(2):
                po = pp.tile([128, 1024], f32, tag="ps")
                for u in range(0, Q, 512):
                    for ki in range(2):
                        nc.tensor.matmul(po[:, u:u + 512], wo_sb[ki][:, mo * 128:(mo + 1) * 128],
                                         avnT[ki][:, u:u + 512], start=(ki == 0), stop=(ki == 1))
                oT = otp.tile([128, S], f32, tag="oT")
                nc.vector.tensor_scalar_add(oT[:, 0:Q], po[:, 0:Q], bo[mo])
                nc.sync.dma_start(out_d[t, mo * 128:(mo + 1) * 128, 0:Q], oT[:, 0:Q])

    nc.compile()
    _PROGRAM_CACHE["nc"] = nc
    return nc


def _prep_inputs(x, w_qkv, b_qkv, w_out, b_out):
    x = np.asarray(x, dtype=np.float32)
    w_qkv = np.asarray(w_qkv, dtype=np.float32)
    b_qkv = np.asarray(b_qkv, dtype=np.float32)
    w_out = np.asarray(w_out, dtype=np.float32)
    b_out = np.asarray(b_out, dtype=np.float32)

    x2 = np.ascontiguousarray(x.reshape(2, N_VIEWS, C, S)).astype(BF16)
    wqkvT_all = np.ascontiguousarray(w_qkv.transpose(0, 2, 1)).astype(BF16)  # [5, C, 3C]
    woT_all = np.ascontiguousarray(w_out.transpose(0, 2, 1)).astype(BF16)    # [5, C, C]
    # fold v-bias into the output bias: o = Wo(avn + bv) + bo
    bo_eff_all = b_out + np.einsum('mce,me->mc', w_out, b_qkv[:, 2 * C:3 * C])

    in_maps = []
    for core in range(N_CORES):
        tasks = ASSIGN[core]
        xq = np.zeros((N_SLOTS, C, S), BF16)
        xkv = np.empty((KV_ROWS, C, S), BF16)
        wqkvT = np.empty((N_SLOTS, C, 3 * C), BF16)
        woT = np.empty((N_SLOTS, C, C), BF16)
        bq = np.empty((N_SLOTS, C, 1), np.float32)
        bo = np.empty((N_SLOTS, C, 1), np.float32)
        for t, (b, i, qh) in enumerate(tasks):
            m = MHA_IDX[i]
            Q = SLOT_Q[t]
            if qh is None:
                xq[t] = x2[b, i]
            else:
                xq[t, :, 0:Q] = x2[b, i][:, qh * Q:(qh + 1) * Q]
            for j, nb in enumerate(SEL[i]):
                xkv[KVOFF[t] + j] = x2[b, nb]
            wqkvT[t] = wqkvT_all[m]
            woT[t] = woT_all[m]
            bq[t, :, 0] = b_qkv[m, 0:C]
            bo[t, :, 0] = bo_eff_all[m]
        in_maps.append({
            "xq": xq, "xkv": xkv, "wqkvT": wqkvT, "woT": woT,
            "bq": bq, "bo": bo,
        })
    return in_maps


def _gather_output(results, dtype):
    y = np.empty((2, N_VIEWS, C, S), np.float32)
    for core in range(N_CORES):
        out = results[core]["out"]
        for t, (b, i, qh) in enumerate(ASSIGN[core]):
            Q = SLOT_Q[t]
            if qh is None:
                y[b, i] = out[t]
            else:
                y[b, i][:, qh * Q:(qh + 1) * Q] = out[t][:, 0:Q]
    return y.reshape(2 * N_VIEWS, C, 32, 32).astype(dtype, copy=False)


def _run(inputs, trace=False, tmpdir=None):
    nc = _build_program()
    in_maps = _prep_inputs(**inputs)
    res = run_bass_kernel_spmd(nc, in_maps, core_ids=list(range(N_CORES)),
                               trace=trace, tmpdir=tmpdir)
    y = _gather_output(res.results, np.asarray(inputs["x"]).dtype)
    return y, res


def kernel(x, w_qkv, b_qkv, w_out, b_out):
    y, _ = _run(dict(x=x, w_qkv=w_qkv, b_qkv=b_qkv, w_out=w_out, b_out=b_out))
    return y
